# revision 6
# baseline (speedup 1.0000x reference)
"""Trainium2 Bass kernel for nn_DeformConv2d_50371376447821 (v4, fp8 DoubleRow).

Per core: one filter group g = core%4, two samples (m*4+g).

Scheme: 3x3 deform-conv as 9 shifted-tap matmuls in fp8e4 (E4M3) using
DoubleRow perf mode (2 K=128 tiles per instruction, 0.5 cyc/row).
Accuracy: per tap out = W8^T X8 + dW8^T X8 (+ W8^T dX8 on the 5 "plus"
taps). Bilinear fold keeps X bounded: per axis/side the ratio
r = min(f,1-f)/max(f,1-f) <= 1 stays on X, m = max(f,1-f) folds into W
on host along with a 2^8 scale (e4m3 denormal avoidance); output is
unscaled by 2^-8 on host. Center-tap X8/dX8 are host-precomputed and
DMA'd. All quantization is RNE-exact vs numpy (verified on HW).
"""

import os
from contextlib import ExitStack

import numpy as np
import ml_dtypes

import concourse.bass as bass
import concourse.bacc as bacc
import concourse.tile as tile
from concourse import mybir
from concourse.bass_utils import run_bass_kernel_spmd

F32 = mybir.dt.float32
BF16 = mybir.dt.bfloat16
FP8 = mybir.dt.float8e4
I32 = mybir.dt.int32
NPBF = ml_dtypes.bfloat16
NPE4 = ml_dtypes.float8_e4m3

KS = 3
H = 36
HP = H + 7
PAD = 3
OG = 256
NSQ = 4
NIMG = 4
NROW = 12
NT = NROW * H  # 432
NN = 3
NWARM = 10
A_W = 8  # host W scale exponent
UNSCALE = np.float32(2.0 ** -A_W)

# plus taps (get X-residual correction); t = i*3+j
PLUS_T = [3, 4, 5, 1, 7]  # (1,0), center, (1,2), (0,1), (2,1)
PT_SLOT = {3: 0, 4: 1, 5: 2, 1: 3, 7: 4}

_CACHED_NC = None
LAST_RESULTS = None


def _build_nc():
    nc = bacc.Bacc("TRN2", target_bir_lowering=False, debug=False, num_devices=8)

    xin = nc.dram_tensor("xin", [128, 2, 2, HP, HP], BF16, kind="ExternalInput").ap()
    filt = nc.dram_tensor("filt", [128, 2, 9, 2, OG], FP8, kind="ExternalInput").ap()
    dfilt = nc.dram_tensor("dfilt", [128, 2, 9, 2, OG], FP8, kind="ExternalInput").ap()
    p8 = nc.dram_tensor("p8", [128, 2, 2, 3, H, H], FP8, kind="ExternalInput").ap()
    dp8 = nc.dram_tensor("dp8", [128, 2, 2, 3, H, H], FP8, kind="ExternalInput").ap()
    params = nc.dram_tensor("params", [128, 2, 16], F32, kind="ExternalInput").ap()
    yout = nc.dram_tensor("yout", [2, OG, H, H], BF16, kind="ExternalOutput").ap()

    XP = nc.alloc_sbuf_tensor("xp", [128, 2, 2, HP, HP], BF16).ap()
    CIQ = nc.alloc_sbuf_tensor("ciq", [128, 2, 2, 2, HP, H], BF16).ap()
    XT = nc.alloc_sbuf_tensor("xt", [128, 2, 2, 2, H, H], BF16).ap()
    X8 = nc.alloc_sbuf_tensor("x8", [128, 2, 2, 9, H, H], FP8).ap()
    DX8 = nc.alloc_sbuf_tensor("dx8", [128, 2, 2, 5, H, H], FP8).ap()
    filt_sb = nc.alloc_sbuf_tensor("filt_sb", [128, 2, 9, 2, OG], FP8).ap()
    dfilt_sb = nc.alloc_sbuf_tensor("dfilt_sb", [128, 2, 9, 2, OG], FP8).ap()
    params_sb = nc.alloc_sbuf_tensor("params_sb", [128, 2, 16], F32).ap()
    wscr = nc.alloc_sbuf_tensor("wscr", [128, 560], BF16).ap()
    OSB = nc.alloc_sbuf_tensor("osb", [128, 2, 2, NN, NT], BF16).ap()

    # params layout per sample: [0:4] f32 ratios (r_row0, r_row2, r_col0,
    # r_col2); [4:12] int32 offsets (rb0, rs0, rb2, rs2, cb0, cs0, cb2, cs2)
    pr = params_sb[:, :, 0:4]
    po = params_sb[:, :, 4:12].bitcast(I32)

    MULT = mybir.AluOpType.mult
    ADD = mybir.AluOpType.add
    COPY = mybir.ActivationFunctionType.Copy
    DVE = mybir.EngineType.DVE
    POOL = mybir.EngineType.Pool
    DR = mybir.MatmulPerfMode.DoubleRow

    chains = {}

    def link(key, instr, reason="order"):
        prev = chains.get(key)
        if prev is not None:
            tile.add_dep_helper(instr.ins, prev.ins, reason=reason)
        chains[key] = instr
        return instr

    def loads(engine, s, cols):
        li, vals = nc.values_load_multi_w_load_instructions(
            po[0:1, s, cols[0]: cols[0] + len(cols)],
            engines=[engine],
            min_val=0,
            max_val=7,
            skip_runtime_bounds_check=True,
        )
        return vals

    with tile.TileContext(nc) as tc, ExitStack() as ctx:
        psum_pool = ctx.enter_context(tc.tile_pool(name="psum", bufs=8, space="PSUM"))

        # ---- PE warmup (ramps clock while DMAs land) ----
        pw = psum_pool.tile([128, NT], F32, tag="ps", name="pw")
        for _ in range(NWARM):
            link("pe", nc.tensor.matmul(
                out=pw[:], lhsT=wscr[:, 0:128], rhs=wscr[:, 128: 128 + NT],
                start=True, stop=True,
            ))

        # ---- DMAs ----
        # SP queue: params first (DVE loads gate stage A), then xin halves.
        nc.sync.dma_start(out=params_sb, in_=params)
        for s in range(2):
            for h in range(2):
                nc.sync.dma_start(out=XP[:, s, h], in_=xin[:, s, h])
        # ACT queue: center pair + filters, s0 before s1.
        for s in range(2):
            nc.scalar.dma_start(out=X8[:, s, :, 3:6], in_=p8[:, s])
            nc.scalar.dma_start(out=DX8[:, s, :, 0:3], in_=dp8[:, s])
            nc.scalar.dma_start(out=filt_sb[:, s], in_=filt[:, s])
            nc.scalar.dma_start(out=dfilt_sb[:, s], in_=dfilt[:, s])

        ctr = slice(PAD, PAD + H)

        for s in range(2):
            # per-sample register loads on the engines that need them
            rb0, rs0, rb2, rs2, cb0, cs0, cb2, cs2 = loads(DVE, s, (0, 1, 2, 3, 4, 5, 6, 7))
            rows = {0: (rb0, rs0, 0), 2: (rb2, rs2, 1)}
            cols = {0: (cb0, cs0, 0), 2: (cb2, cs2, 1)}

            # ---- DVE: stage A column interp (bf16), per (j, h) ----
            for j in (0, 2):
                cb, cs, jj = cols[j]
                rj = pr[:, s, 2 + jj: 3 + jj]
                for h in range(2):
                    link("dve", nc.vector.scalar_tensor_tensor(
                        out=CIQ[:, s, h, jj],
                        in0=XP[:, s, h, :, bass.ds(cs, H)],
                        scalar=rj,
                        in1=XP[:, s, h, :, bass.ds(cb, H)],
                        op0=MULT, op1=ADD,
                    ))

            # ---- corners: fused STT -> fp8, per half, all on DVE ----
            corner_list = [(0, 0), (0, 2), (2, 0), (2, 2)]
            for i, j in corner_list:
                t = i * 3 + j
                jj = cols[j][2]
                rb, rs, ii = rows[i]
                ri = pr[:, s, ii: ii + 1]
                for h in range(2):
                    link("dve", nc.vector.scalar_tensor_tensor(
                        out=X8[:, s, h, t],
                        in0=CIQ[:, s, h, jj, bass.ds(rs, H), :],
                        scalar=ri,
                        in1=CIQ[:, s, h, jj, bass.ds(rb, H), :],
                        op0=MULT, op1=ADD,
                    ))

            # ---- (i,1): row-interp Xt (DVE), cast (ACT), resid (Pool TT) ----
            for i in (0, 2):
                rb, rs, ii = rows[i]
                ri = pr[:, s, ii: ii + 1]
                for h in range(2):
                    link("dve", nc.vector.scalar_tensor_tensor(
                        out=XT[:, s, h, ii],
                        in0=XP[:, s, h, bass.ds(rs, H), ctr],
                        scalar=ri,
                        in1=XP[:, s, h, bass.ds(rb, H), ctr],
                        op0=MULT, op1=ADD,
                    ))
            for i in (0, 2):
                ii = rows[i][2]
                t = i * 3 + 1
                for h in range(2):
                    link("act", nc.scalar.activation(
                        out=X8[:, s, h, t], in_=XT[:, s, h, ii], func=COPY,
                    ))
            for i in (0, 2):
                t = i * 3 + 1
                ii = rows[i][2]
                for h in range(2):
                    link("pool", nc.gpsimd.tensor_tensor(
                        out=DX8[:, s, h, PT_SLOT[t]],
                        in0=XT[:, s, h, ii],
                        in1=X8[:, s, h, t],
                        op=mybir.AluOpType.subtract,
                    ))

            # ---- PE: fp8 DoubleRow matmuls ----
            psums = [[psum_pool.tile([128, NT], F32, tag="ps", name=f"ps_{s}_{o}_{n}")
                      for n in range(NN)] for o in range(2)]

            def mm(kind, t, o, n, start=False, stop=False):
                if kind == "m":
                    lhsT = filt_sb[:, s, t, :, o * 128: (o + 1) * 128]
                    rhs = X8[:, s, :, t, n * NROW: (n + 1) * NROW, :]
                elif kind == "w":
                    lhsT = dfilt_sb[:, s, t, :, o * 128: (o + 1) * 128]
                    rhs = X8[:, s, :, t, n * NROW: (n + 1) * NROW, :]
                else:
                    lhsT = filt_sb[:, s, t, :, o * 128: (o + 1) * 128]
                    rhs = DX8[:, s, :, PT_SLOT[t], n * NROW: (n + 1) * NROW, :]
                link("pe", nc.tensor.matmul(
                    out=psums[o][n][:], lhsT=lhsT, rhs=rhs,
                    start=start, stop=stop, perf_mode=DR,
                ))

            # phase order matched to producer readiness
            phases = [("m", 4), ("x", 4), ("w", 4),
                      ("m", 3), ("w", 3), ("m", 5), ("w", 5),
                      ("m", 0), ("w", 0), ("m", 2), ("w", 2),
                      ("m", 6), ("w", 6), ("m", 8), ("w", 8),
                      ("m", 1), ("w", 1), ("m", 7), ("w", 7),
                      ("x", 3), ("x", 5), ("x", 1)]
            for kind, t in phases:
                for o in range(2):
                    for n in range(NN):
                        mm(kind, t, o, n, start=(kind == "m" and t == 4))
            # final phase psum-major with stop, then evac + out DMA
            evac = [("act", "sp"), ("dve", "sp"), ("act", "actq"),
                    ("act", "sp"), ("dve", "actq"), ("act", "sp")]
            gi = 0
            for o in range(2):
                for n in range(NN):
                    mm("x", 7, o, n, stop=True)
                    ev, dq = evac[gi]
                    gi += 1
                    osl = OSB[:, s, o, n]
                    ysl = yout[s, o * 128: (o + 1) * 128, n * NROW: (n + 1) * NROW, :]
                    if ev == "act":
                        link("act", nc.scalar.activation(
                            out=osl, in_=psums[o][n][:], func=COPY, scale=float(UNSCALE),
                        ))
                    else:
                        link("dve", nc.vector.tensor_scalar(
                            out=osl, in0=psums[o][n][:],
                            scalar1=float(UNSCALE), scalar2=None, op0=MULT,
                        ))
                    (nc.sync if dq == "sp" else nc.scalar).dma_start(out=ysl, in_=osl)

    if not nc.is_finalized():
        nc.finalize()
    return nc


def _get_nc():
    global _CACHED_NC
    if _CACHED_NC is None:
        _CACHED_NC = _build_nc()
    return _CACHED_NC


def _side_params(off_axis):
    out = {}
    for i, sign in ((0, np.float32(-1.0)), (2, np.float32(1.0))):
        d = sign * (np.float32(KS) / np.float32(off_axis))
        D = int(np.floor(d))
        f = np.float32(d - np.float32(D))
        if f <= 0.5:
            base, scaled = D, D + 1
            r = np.float32(f / (np.float32(1.0) - f))
            m = np.float32(1.0) - f
        else:
            base, scaled = D + 1, D
            r = np.float32((np.float32(1.0) - f) / f)
            m = f
        out[i] = (base + PAD, scaled + PAD, r, m)
    return out


def kernel(x, target_filter, offset):
    x = np.ascontiguousarray(np.asarray(x, dtype=np.float32))
    tf = np.ascontiguousarray(np.asarray(target_filter, dtype=np.float32))
    offset = np.asarray(offset, dtype=np.float32)

    nc = _get_nc()

    in_maps = []
    core_meta = []
    for k in range(8):
        g = k % 4
        ms = (2 * (k // 4), 2 * (k // 4) + 1)
        W = tf[g * OG: (g + 1) * OG].reshape(OG, 2, 128, KS, KS)  # [o,h,p,i,j]

        xin = np.zeros((128, 2, 2, HP, HP), NPBF)
        filt8 = np.zeros((128, 2, 9, 2, OG), NPE4)
        dfilt8 = np.zeros((128, 2, 9, 2, OG), NPE4)
        p8a = np.zeros((128, 2, 2, 3, H, H), NPE4)
        dp8a = np.zeros((128, 2, 2, 3, H, H), NPE4)
        params = np.zeros((128, 2, 16), np.float32)

        for si, m in enumerate(ms):
            b = m * NSQ + g
            xs = x[m, g]
            rows = _side_params(offset[b][0])
            cols = _side_params(offset[b][1])
            for h in range(2):
                xbf = xs[h * 128: (h + 1) * 128].astype(NPBF)
                xin[:, si, h, PAD: PAD + H, PAD: PAD + H] = xbf
                xpad = np.asarray(xin[:, si, h], NPBF).astype(np.float32)
                # host-shipped X8/dX8 pairs for taps (1,0), center, (1,2)
                for sl, j in ((0, 0), (2, 2)):
                    cb, cs, rj, mj = cols[j]
                    xt = (rj * xpad[:, PAD: PAD + H, cs: cs + H]
                          + xpad[:, PAD: PAD + H, cb: cb + H]).astype(NPBF)
                    xq = xt.astype(NPE4)
                    p8a[:, si, h, sl] = xq
                    dp8a[:, si, h, sl] = (xt.astype(np.float32)
                                          - xq.astype(np.float32)).astype(NPE4)
                xq = xbf.astype(NPE4)
                p8a[:, si, h, 1] = xq
                dp8a[:, si, h, 1] = (xbf.astype(np.float32)
                                     - xq.astype(np.float32)).astype(NPE4)
            pvals = np.zeros(16, np.float32)
            pvals[0:2] = [rows[0][2], rows[2][2]]
            pvals[2:4] = [cols[0][2], cols[2][2]]
            off_i = np.array([rows[0][0], rows[0][1], rows[2][0], rows[2][1],
                              cols[0][0], cols[0][1], cols[2][0], cols[2][1]],
                             np.int32)
            assert off_i.min() >= 0 and off_i.max() <= 7, off_i
            pvals[4:12] = off_i.view(np.float32)
            params[:, si, :] = pvals[None]

            for i in range(3):
                for j in range(3):
                    t = i * 3 + j
                    mi = rows[i][3] if i != 1 else np.float32(1.0)
                    mj = cols[j][3] if j != 1 else np.float32(1.0)
                    sc = np.float32(mi * mj * (1 << A_W))
                    for h in range(2):
                        wf = W[:, h, :, i, j].T.astype(np.float32) * sc  # [p, o]
                        w8 = wf.astype(NPE4)
                        filt8[:, si, t, h, :] = w8
                        dfilt8[:, si, t, h, :] = (
                            wf - w8.astype(np.float32)).astype(NPE4)

        in_maps.append({"xin": xin, "filt": filt8, "dfilt": dfilt8,
                        "p8": p8a, "dp8": dp8a, "params": params})
        core_meta.append((g, ms))

    trace = bool(int(os.environ.get("KERNEL_TRACE", "0")))
    res = None
    last_exc = None
    for attempt in range(3):
        try:
            res = run_bass_kernel_spmd(
                nc, in_maps, list(range(8)), trace=trace and attempt == 0
            )
            break
        except Exception as exc:
            last_exc = exc
    if res is None:
        raise last_exc
    global LAST_RESULTS
    LAST_RESULTS = res

    out = np.empty((NIMG, NSQ * OG, H, H), np.float32)
    for k in range(8):
        g, ms = core_meta[k]
        y = res.results[k]["yout"]
        for si, m in enumerate(ms):
            out[m, g * OG: (g + 1) * OG] = np.asarray(y[si]).astype(np.float32)
    return out


# revision 7
# speedup vs baseline: 1.6826x; 1.6826x over previous
"""Trainium2 Bass kernel for nn_DeformConv2d_50371376447821 (v5, fp8 DoubleRow).

Per core: one filter group g = core%4, two samples. Host precomputes all
9 bilinear tap images quantized to fp8e4 (X8) plus fp8 residuals (dX8)
for the 5 high-variance taps, and folded/scaled fp8 filters W8 + dW8.
Device: pure DoubleRow fp8 matmul accumulation (per tap: W8^T X8 +
dW8^T X8 [+ W8^T dX8]) into 6 psums per sample, evac with 2^-8 unscale.
"""

import os
from contextlib import ExitStack

import numpy as np
import ml_dtypes

import concourse.bass as bass
import concourse.bacc as bacc
import concourse.tile as tile
from concourse import mybir
from concourse.bass_utils import run_bass_kernel_spmd

F32 = mybir.dt.float32
BF16 = mybir.dt.bfloat16
FP8 = mybir.dt.float8e4
NPBF = ml_dtypes.bfloat16
NPE4 = ml_dtypes.float8_e4m3

KS = 3
H = 36
HP = H + 7
PAD = 3
OG = 256
NSQ = 4
NIMG = 4
NROW = 12
NT = NROW * H
NN = 3
NWARM = 10
A_W = 8
UNSCALE = float(np.float32(2.0 ** -A_W))

# tap positions in consumption order; X8/filt tensors use POS indexing
TORD = [4, 3, 5, 0, 2, 6, 8, 1, 7]
T2POS = {t: p for p, t in enumerate(TORD)}
XRES_T = [4, 3, 5, 1, 7]  # taps with X-residual, DX8 slot = index here

_CACHED_NC = None
LAST_RESULTS = None


def _build_nc():
    nc = bacc.Bacc("TRN2", target_bir_lowering=False, debug=False, num_devices=8)

    p8 = nc.dram_tensor("p8", [128, 2, 2, 9, H, H], FP8, kind="ExternalInput").ap()
    dp8 = nc.dram_tensor("dp8", [128, 2, 2, 5, H, H], FP8, kind="ExternalInput").ap()
    filt = nc.dram_tensor("filt", [128, 2, 9, 2, OG], FP8, kind="ExternalInput").ap()
    dfilt = nc.dram_tensor("dfilt", [128, 2, 9, 2, OG], FP8, kind="ExternalInput").ap()
    yout = nc.dram_tensor("yout", [2, OG, H, H], BF16, kind="ExternalOutput").ap()

    X8 = nc.alloc_sbuf_tensor("x8", [128, 2, 2, 9, H, H], FP8).ap()
    DX8 = nc.alloc_sbuf_tensor("dx8", [128, 2, 2, 5, H, H], FP8).ap()
    filt_sb = nc.alloc_sbuf_tensor("filt_sb", [128, 2, 9, 2, OG], FP8).ap()
    dfilt_sb = nc.alloc_sbuf_tensor("dfilt_sb", [128, 2, 9, 2, OG], FP8).ap()
    wscr = nc.alloc_sbuf_tensor("wscr", [128, 560], BF16).ap()
    OSB = nc.alloc_sbuf_tensor("osb", [128, 2, 2, NN, NT], BF16).ap()

    MULT = mybir.AluOpType.mult
    COPY = mybir.ActivationFunctionType.Copy
    DR = mybir.MatmulPerfMode.DoubleRow

    chains = {}

    def link(key, instr, reason="order"):
        prev = chains.get(key)
        if prev is not None:
            tile.add_dep_helper(instr.ins, prev.ins, reason=reason)
        chains[key] = instr
        return instr

    with tile.TileContext(nc) as tc, ExitStack() as ctx:
        psum_pool = ctx.enter_context(tc.tile_pool(name="psum", bufs=8, space="PSUM"))

        pw = psum_pool.tile([128, NT], F32, tag="ps", name="pw")
        for _ in range(NWARM):
            link("pe", nc.tensor.matmul(
                out=pw[:], lhsT=wscr[:, 0:128], rhs=wscr[:, 128: 128 + NT],
                start=True, stop=True,
            ))

        # DMA order = PE consumption order. SP queue: X8 chunks; ACT queue:
        # filters + residuals.
        for s in range(2):
            nc.sync.dma_start(out=X8[:, s, :, 0:3], in_=p8[:, s, :, 0:3])
            nc.scalar.dma_start(out=filt_sb[:, s], in_=filt[:, s])
            nc.sync.dma_start(out=X8[:, s, :, 3:9], in_=p8[:, s, :, 3:9])
            nc.scalar.dma_start(out=dfilt_sb[:, s], in_=dfilt[:, s])
            nc.sync.dma_start(out=DX8[:, s], in_=dp8[:, s])

        for s in range(2):
            psums = [[psum_pool.tile([128, NT], F32, tag="ps", name=f"ps_{s}_{o}_{n}")
                      for n in range(NN)] for o in range(2)]

            def mm(kind, t, o, n, start=False, stop=False):
                pos = T2POS[t]
                if kind == "w":
                    lhsT = dfilt_sb[:, s, pos, :, o * 128: (o + 1) * 128]
                else:
                    lhsT = filt_sb[:, s, pos, :, o * 128: (o + 1) * 128]
                if kind == "x":
                    rhs = DX8[:, s, :, XRES_T.index(t), n * NROW: (n + 1) * NROW, :]
                else:
                    rhs = X8[:, s, :, pos, n * NROW: (n + 1) * NROW, :]
                link("pe", nc.tensor.matmul(
                    out=psums[o][n][:], lhsT=lhsT, rhs=rhs,
                    start=start, stop=stop, perf_mode=DR,
                ))

            phases = [("m", t) for t in TORD] + [("w", t) for t in TORD] \
                + [("x", 4), ("x", 3), ("x", 5), ("x", 1)]
            for kind, t in phases:
                for o in range(2):
                    for n in range(NN):
                        mm(kind, t, o, n, start=(kind == "m" and t == 4))
            evac = [("act", "sp"), ("dve", "sp"), ("act", "actq"),
                    ("act", "sp"), ("dve", "actq"), ("act", "sp")]
            gi = 0
            for o in range(2):
                for n in range(NN):
                    mm("x", 7, o, n, stop=True)
                    ev, dq = evac[gi]
                    gi += 1
                    osl = OSB[:, s, o, n]
                    ysl = yout[s, o * 128: (o + 1) * 128, n * NROW: (n + 1) * NROW, :]
                    if ev == "act":
                        link("act", nc.scalar.activation(
                            out=osl, in_=psums[o][n][:], func=COPY, scale=UNSCALE,
                        ))
                    else:
                        link("dve", nc.vector.tensor_scalar(
                            out=osl, in0=psums[o][n][:],
                            scalar1=UNSCALE, scalar2=None, op0=MULT,
                        ))
                    (nc.sync if dq == "sp" else nc.scalar).dma_start(out=ysl, in_=osl)

    if not nc.is_finalized():
        nc.finalize()
    return nc


def _get_nc():
    global _CACHED_NC
    if _CACHED_NC is None:
        _CACHED_NC = _build_nc()
    return _CACHED_NC


def _side_params(off_axis):
    out = {}
    for i, sign in ((0, np.float32(-1.0)), (2, np.float32(1.0))):
        d = sign * (np.float32(KS) / np.float32(off_axis))
        D = int(np.floor(d))
        f = np.float32(d - np.float32(D))
        if f <= 0.5:
            base, scaled = D, D + 1
            r = np.float32(f / (np.float32(1.0) - f))
            m = np.float32(1.0) - f
        else:
            base, scaled = D + 1, D
            r = np.float32((np.float32(1.0) - f) / f)
            m = f
        out[i] = (base + PAD, scaled + PAD, r, m)
    return out


def kernel(x, target_filter, offset):
    x = np.ascontiguousarray(np.asarray(x, dtype=np.float32))
    tf = np.ascontiguousarray(np.asarray(target_filter, dtype=np.float32))
    offset = np.asarray(offset, dtype=np.float32)

    nc = _get_nc()
    ctr = slice(PAD, PAD + H)

    in_maps = []
    core_meta = []
    for k in range(8):
        g = k % 4
        ms = (2 * (k // 4), 2 * (k // 4) + 1)
        W = tf[g * OG: (g + 1) * OG].reshape(OG, 2, 128, KS, KS)

        p8a = np.zeros((128, 2, 2, 9, H, H), NPE4)
        dp8a = np.zeros((128, 2, 2, 5, H, H), NPE4)
        filt8 = np.zeros((128, 2, 9, 2, OG), NPE4)
        dfilt8 = np.zeros((128, 2, 9, 2, OG), NPE4)

        for si, m in enumerate(ms):
            b = m * NSQ + g
            rows = _side_params(offset[b][0])
            cols = _side_params(offset[b][1])
            for h in range(2):
                xbf = x[m, g, h * 128: (h + 1) * 128].astype(NPBF)
                xp = np.zeros((128, HP, HP), np.float32)
                xp[:, ctr, ctr] = xbf.astype(np.float32)
                CIQ = {}
                for j in (0, 2):
                    cb, cs, rj, mj = cols[j]
                    CIQ[j] = (rj * xp[:, :, cs:cs + H] + xp[:, :, cb:cb + H]
                              ).astype(NPBF).astype(np.float32)
                for i in range(3):
                    for j in range(3):
                        t = i * 3 + j
                        if i == 1 and j == 1:
                            xt = xbf.astype(np.float32)
                        elif i == 1:
                            xt = CIQ[j][:, ctr, :]
                        elif j == 1:
                            rb, rs, ri, mi = rows[i]
                            xt = (ri * xp[:, rs:rs + H, ctr]
                                  + xp[:, rb:rb + H, ctr]).astype(NPBF
                                  ).astype(np.float32)
                        else:
                            rb, rs, ri, mi = rows[i]
                            xt = (ri * CIQ[j][:, rs:rs + H, :]
                                  + CIQ[j][:, rb:rb + H, :]).astype(NPBF
                                  ).astype(np.float32)
                        xq = xt.astype(NPE4)
                        p8a[:, si, h, T2POS[t]] = xq
                        if t in XRES_T:
                            dp8a[:, si, h, XRES_T.index(t)] = (
                                xt - xq.astype(np.float32)).astype(NPE4)
            for i in range(3):
                for j in range(3):
                    t = i * 3 + j
                    mi = rows[i][3] if i != 1 else np.float32(1.0)
                    mj = cols[j][3] if j != 1 else np.float32(1.0)
                    sc = np.float32(mi * mj * (1 << A_W))
                    for h in range(2):
                        wf = W[:, h, :, i, j].T.astype(np.float32) * sc
                        w8 = wf.astype(NPE4)
                        filt8[:, si, T2POS[t], h, :] = w8
                        dfilt8[:, si, T2POS[t], h, :] = (
                            wf - w8.astype(np.float32)).astype(NPE4)

        in_maps.append({"p8": p8a, "dp8": dp8a, "filt": filt8, "dfilt": dfilt8})
        core_meta.append((g, ms))

    trace = bool(int(os.environ.get("KERNEL_TRACE", "0")))
    res = None
    last_exc = None
    for attempt in range(3):
        try:
            res = run_bass_kernel_spmd(
                nc, in_maps, list(range(8)), trace=trace and attempt == 0
            )
            break
        except Exception as exc:
            last_exc = exc
    if res is None:
        raise last_exc
    global LAST_RESULTS
    LAST_RESULTS = res

    out = np.empty((NIMG, NSQ * OG, H, H), np.float32)
    for k in range(8):
        g, ms = core_meta[k]
        y = res.results[k]["yout"]
        for si, m in enumerate(ms):
            out[m, g * OG: (g + 1) * OG] = np.asarray(y[si]).astype(np.float32)
    return out


# revision 9
# speedup vs baseline: 1.7308x; 1.0286x over previous
"""Trainium2 Bass kernel for nn_DeformConv2d_50371376447821 (v5, fp8 DoubleRow).

Per core: one filter group g = core%4, two samples. Host precomputes all
9 bilinear tap images quantized to fp8e4 (X8) plus fp8 residuals (dX8)
for the 5 high-variance taps, and folded/scaled fp8 filters W8 + dW8.
Device: pure DoubleRow fp8 matmul accumulation (per tap: W8^T X8 +
dW8^T X8 [+ W8^T dX8]) into 6 psums per sample, evac with 2^-8 unscale.
"""

import os
from contextlib import ExitStack

import numpy as np
import ml_dtypes

import concourse.bass as bass
import concourse.bacc as bacc
import concourse.tile as tile
from concourse import mybir
from concourse.bass_utils import run_bass_kernel_spmd

F32 = mybir.dt.float32
BF16 = mybir.dt.bfloat16
FP8 = mybir.dt.float8e4
NPBF = ml_dtypes.bfloat16
NPE4 = ml_dtypes.float8_e4m3

KS = 3
H = 36
HP = H + 7
PAD = 3
OG = 256
NSQ = 4
NIMG = 4
NROW = 12
NT = NROW * H
NN = 3
NWARM = 10
A_W = 8
UNSCALE = float(np.float32(2.0 ** -A_W))

# tap positions in consumption order; X8/filt tensors use POS indexing
TORD = [4, 3, 5, 0, 2, 6, 8, 1, 7]
T2POS = {t: p for p, t in enumerate(TORD)}
XRES_T = [4, 3, 5, 1, 7]  # taps with X-residual, DX8 slot = index here

_CACHED_NC = None
LAST_RESULTS = None


def _build_nc():
    nc = bacc.Bacc("TRN2", target_bir_lowering=False, debug=False, num_devices=8)

    p8 = nc.dram_tensor("p8", [128, 2, 2, 9, H, H], FP8, kind="ExternalInput").ap()
    dp8 = nc.dram_tensor("dp8", [128, 2, 2, 5, H, H], FP8, kind="ExternalInput").ap()
    filt = nc.dram_tensor("filt", [128, 2, 9, 2, OG], FP8, kind="ExternalInput").ap()
    dfilt = nc.dram_tensor("dfilt", [128, 2, 9, 2, OG], FP8, kind="ExternalInput").ap()
    yout = nc.dram_tensor("yout", [2, OG, H, H], BF16, kind="ExternalOutput").ap()

    X8 = nc.alloc_sbuf_tensor("x8", [128, 2, 2, 9, H, H], FP8).ap()
    DX8 = nc.alloc_sbuf_tensor("dx8", [128, 2, 2, 5, H, H], FP8).ap()
    filt_sb = nc.alloc_sbuf_tensor("filt_sb", [128, 2, 9, 2, OG], FP8).ap()
    dfilt_sb = nc.alloc_sbuf_tensor("dfilt_sb", [128, 2, 9, 2, OG], FP8).ap()
    wscr = nc.alloc_sbuf_tensor("wscr", [128, 560], BF16).ap()
    OSB = nc.alloc_sbuf_tensor("osb", [128, 2, 2, NN, NT], BF16).ap()

    MULT = mybir.AluOpType.mult
    COPY = mybir.ActivationFunctionType.Copy
    DR = mybir.MatmulPerfMode.DoubleRow

    chains = {}

    def link(key, instr, reason="order"):
        prev = chains.get(key)
        if prev is not None:
            tile.add_dep_helper(instr.ins, prev.ins, reason=reason)
        chains[key] = instr
        return instr

    with tile.TileContext(nc) as tc, ExitStack() as ctx:
        psum_pool = ctx.enter_context(tc.tile_pool(name="psum", bufs=8, space="PSUM"))

        pw = psum_pool.tile([128, NT], F32, tag="ps", name="pw")
        for _ in range(NWARM):
            link("pe", nc.tensor.matmul(
                out=pw[:], lhsT=wscr[:, 0:128], rhs=wscr[:, 128: 128 + NT],
                start=True, stop=True,
            ))

        # DMA order = PE consumption order. SP queue: X8 chunks; ACT queue:
        # filters + residuals.
        # s0 on SP/ACT queues; s1 on DVE/Pool queues so the per-sample
        # output DMAs (SP/ACT) are not stuck behind s1 inputs.
        nc.sync.dma_start(out=X8[:, 0, :, 0:3], in_=p8[:, 0, :, 0:3])
        nc.scalar.dma_start(out=filt_sb[:, 0], in_=filt[:, 0])
        nc.sync.dma_start(out=X8[:, 0, :, 3:9], in_=p8[:, 0, :, 3:9])
        nc.scalar.dma_start(out=dfilt_sb[:, 0], in_=dfilt[:, 0])
        nc.sync.dma_start(out=DX8[:, 0], in_=dp8[:, 0])
        nc.gpsimd.dma_start(out=X8[:, 1, :, 0:3], in_=p8[:, 1, :, 0:3])
        nc.gpsimd.dma_start(out=filt_sb[:, 1], in_=filt[:, 1])
        nc.gpsimd.dma_start(out=X8[:, 1, :, 3:9], in_=p8[:, 1, :, 3:9])
        nc.gpsimd.dma_start(out=dfilt_sb[:, 1], in_=dfilt[:, 1])
        nc.gpsimd.dma_start(out=DX8[:, 1], in_=dp8[:, 1])

        for s in range(2):
            psums = [[psum_pool.tile([128, NT], F32, tag="ps", name=f"ps_{s}_{o}_{n}")
                      for n in range(NN)] for o in range(2)]

            def mm(kind, t, o, n, start=False, stop=False):
                pos = T2POS[t]
                if kind == "w":
                    lhsT = dfilt_sb[:, s, pos, :, o * 128: (o + 1) * 128]
                else:
                    lhsT = filt_sb[:, s, pos, :, o * 128: (o + 1) * 128]
                if kind == "x":
                    rhs = DX8[:, s, :, XRES_T.index(t), n * NROW: (n + 1) * NROW, :]
                else:
                    rhs = X8[:, s, :, pos, n * NROW: (n + 1) * NROW, :]
                link("pe", nc.tensor.matmul(
                    out=psums[o][n][:], lhsT=lhsT, rhs=rhs,
                    start=start, stop=stop, perf_mode=DR,
                ))

            phases = [("m", t) for t in TORD] + [("w", t) for t in TORD] \
                + [("x", 4), ("x", 3), ("x", 5), ("x", 1)]
            for kind, t in phases:
                for o in range(2):
                    for n in range(NN):
                        mm(kind, t, o, n, start=(kind == "m" and t == 4))
            evac = [("act", "sp"), ("dve", "sp"), ("act", "actq"),
                    ("act", "sp"), ("dve", "actq"), ("act", "sp")]
            gi = 0
            for o in range(2):
                for n in range(NN):
                    mm("x", 7, o, n, stop=True)
                    ev, dq = evac[gi]
                    gi += 1
                    osl = OSB[:, s, o, n]
                    ysl = yout[s, o * 128: (o + 1) * 128, n * NROW: (n + 1) * NROW, :]
                    if ev == "act":
                        link("act", nc.scalar.activation(
                            out=osl, in_=psums[o][n][:], func=COPY, scale=UNSCALE,
                        ))
                    else:
                        link("dve", nc.vector.tensor_scalar(
                            out=osl, in0=psums[o][n][:],
                            scalar1=UNSCALE, scalar2=None, op0=MULT,
                        ))
                    (nc.sync if dq == "sp" else nc.scalar).dma_start(out=ysl, in_=osl)

    if not nc.is_finalized():
        nc.finalize()
    return nc


def _get_nc():
    global _CACHED_NC
    if _CACHED_NC is None:
        _CACHED_NC = _build_nc()
    return _CACHED_NC


def _side_params(off_axis):
    out = {}
    for i, sign in ((0, np.float32(-1.0)), (2, np.float32(1.0))):
        d = sign * (np.float32(KS) / np.float32(off_axis))
        D = int(np.floor(d))
        f = np.float32(d - np.float32(D))
        if f <= 0.5:
            base, scaled = D, D + 1
            r = np.float32(f / (np.float32(1.0) - f))
            m = np.float32(1.0) - f
        else:
            base, scaled = D + 1, D
            r = np.float32((np.float32(1.0) - f) / f)
            m = f
        out[i] = (base + PAD, scaled + PAD, r, m)
    return out


def kernel(x, target_filter, offset):
    x = np.ascontiguousarray(np.asarray(x, dtype=np.float32))
    tf = np.ascontiguousarray(np.asarray(target_filter, dtype=np.float32))
    offset = np.asarray(offset, dtype=np.float32)

    nc = _get_nc()
    ctr = slice(PAD, PAD + H)

    in_maps = []
    core_meta = []
    for k in range(8):
        g = k % 4
        ms = (2 * (k // 4), 2 * (k // 4) + 1)
        W = tf[g * OG: (g + 1) * OG].reshape(OG, 2, 128, KS, KS)

        p8a = np.zeros((128, 2, 2, 9, H, H), NPE4)
        dp8a = np.zeros((128, 2, 2, 5, H, H), NPE4)
        filt8 = np.zeros((128, 2, 9, 2, OG), NPE4)
        dfilt8 = np.zeros((128, 2, 9, 2, OG), NPE4)

        for si, m in enumerate(ms):
            b = m * NSQ + g
            rows = _side_params(offset[b][0])
            cols = _side_params(offset[b][1])
            for h in range(2):
                xbf = x[m, g, h * 128: (h + 1) * 128].astype(NPBF)
                xp = np.zeros((128, HP, HP), np.float32)
                xp[:, ctr, ctr] = xbf.astype(np.float32)
                CIQ = {}
                for j in (0, 2):
                    cb, cs, rj, mj = cols[j]
                    CIQ[j] = (rj * xp[:, :, cs:cs + H] + xp[:, :, cb:cb + H]
                              ).astype(NPBF).astype(np.float32)
                for i in range(3):
                    for j in range(3):
                        t = i * 3 + j
                        if i == 1 and j == 1:
                            xt = xbf.astype(np.float32)
                        elif i == 1:
                            xt = CIQ[j][:, ctr, :]
                        elif j == 1:
                            rb, rs, ri, mi = rows[i]
                            xt = (ri * xp[:, rs:rs + H, ctr]
                                  + xp[:, rb:rb + H, ctr]).astype(NPBF
                                  ).astype(np.float32)
                        else:
                            rb, rs, ri, mi = rows[i]
                            xt = (ri * CIQ[j][:, rs:rs + H, :]
                                  + CIQ[j][:, rb:rb + H, :]).astype(NPBF
                                  ).astype(np.float32)
                        xq = xt.astype(NPE4)
                        p8a[:, si, h, T2POS[t]] = xq
                        if t in XRES_T:
                            dp8a[:, si, h, XRES_T.index(t)] = (
                                xt - xq.astype(np.float32)).astype(NPE4)
            for i in range(3):
                for j in range(3):
                    t = i * 3 + j
                    mi = rows[i][3] if i != 1 else np.float32(1.0)
                    mj = cols[j][3] if j != 1 else np.float32(1.0)
                    sc = np.float32(mi * mj * (1 << A_W))
                    for h in range(2):
                        wf = W[:, h, :, i, j].T.astype(np.float32) * sc
                        w8 = wf.astype(NPE4)
                        filt8[:, si, T2POS[t], h, :] = w8
                        dfilt8[:, si, T2POS[t], h, :] = (
                            wf - w8.astype(np.float32)).astype(NPE4)

        in_maps.append({"p8": p8a, "dp8": dp8a, "filt": filt8, "dfilt": dfilt8})
        core_meta.append((g, ms))

    trace = bool(int(os.environ.get("KERNEL_TRACE", "0")))
    res = None
    last_exc = None
    for attempt in range(3):
        try:
            res = run_bass_kernel_spmd(
                nc, in_maps, list(range(8)), trace=trace and attempt == 0
            )
            break
        except Exception as exc:
            last_exc = exc
    if res is None:
        raise last_exc
    global LAST_RESULTS
    LAST_RESULTS = res

    out = np.empty((NIMG, NSQ * OG, H, H), np.float32)
    for k in range(8):
        g, ms = core_meta[k]
        y = res.results[k]["yout"]
        for si, m in enumerate(ms):
            out[m, g * OG: (g + 1) * OG] = np.asarray(y[si]).astype(np.float32)
    return out


# revision 10
# speedup vs baseline: 1.7746x; 1.0253x over previous
"""Trainium2 Bass kernel for nn_DeformConv2d_50371376447821 (v3, bf16, static schedule).

Per core: one filter group g = core%4, two samples (m*4+g).
Host folds per-sample bilinear "lo" weights into two bf16 filter copies
(kt rows in tap-consumption order) and ships pre-padded bf16 inputs.

Device schedule (all per-engine queues explicitly chained):
  PE:   warmup stream (scratch) ramps the clock; then per sample, taps in
        order [4,3,5,1,0,2,7,6,8] o-interleaved; final two taps grouped
        per (o,n) with staggered stops feeding evac+DMA pipelines.
  DVE:  stage A col-interp (TS 4x + TT 2x per (j,b)), stage B row-interp
        (TS+TT per (islab,jpair)); s0 then s1.
  ACT:  j=1 scaled copies; all s0 evacs + half of s1 evacs (psum->bf16).
  Pool: j=1 lo adds.
  DMA:  SP queue: xin (s0b0 split in 2) then output chunks; ACT queue:
        filter chunks most-urgent first; Pool SWDGE: params.
"""

import os
from contextlib import ExitStack

import numpy as np
import ml_dtypes

import concourse.bass as bass
import concourse.bacc as bacc
import concourse.tile as tile
from concourse import mybir
from concourse.bass_utils import run_bass_kernel_spmd

F32 = mybir.dt.float32
BF16 = mybir.dt.bfloat16
I32 = mybir.dt.int32
NPBF = ml_dtypes.bfloat16

KS = 3
H = 36
HP = H + 7
PAD = 3
OG = 256
NSQ = 4
NIMG = 4
NROW = 12
NT = NROW * H   # 432
NN = 3
TAPORD = [4, 3, 5, 1, 0, 2, 7, 6, 8]
NWARM = 10

_CACHED_NC = None
LAST_RESULTS = None


def _build_nc():
    nc = bacc.Bacc("TRN2", target_bir_lowering=False, debug=False, num_devices=8)

    xin = nc.dram_tensor("xin", [128, 2, 2, HP, HP], BF16, kind="ExternalInput").ap()
    filt = nc.dram_tensor("filt", [128, 2, 18, OG], BF16, kind="ExternalInput").ap()
    params = nc.dram_tensor("params", [128, 2, 8], F32, kind="ExternalInput").ap()
    yout = nc.dram_tensor("yout", [2, OG, H, H], BF16, kind="ExternalOutput").ap()

    XP = [nc.alloc_sbuf_tensor(f"xp_{s}", [128, 2, HP, HP], BF16).ap() for s in range(2)]
    CI = [nc.alloc_sbuf_tensor(f"ci_{s}", [128, 4, HP, H], BF16).ap() for s in range(2)]
    XO = [nc.alloc_sbuf_tensor(f"xo_{s}", [128, 2, 6, H, H], BF16).ap() for s in range(2)]
    filt_sb = nc.alloc_sbuf_tensor("filt_sb", [128, 2, 18, OG], BF16).ap()
    params_sb = nc.alloc_sbuf_tensor("params_sb", [128, 2, 8], F32).ap()
    pratio_sb = params_sb[:, :, 0:4]
    poff_sb = params_sb[:, :, 4:8].bitcast(I32)
    wscr = nc.alloc_sbuf_tensor("wscr", [128, 560], BF16).ap()
    OSB = [nc.alloc_sbuf_tensor(f"osb_{s}", [128, 2, NN, NT], BF16).ap() for s in range(2)]

    MULT = mybir.AluOpType.mult
    ADD = mybir.AluOpType.add
    COPY = mybir.ActivationFunctionType.Copy
    DVE = mybir.EngineType.DVE
    ACT = mybir.EngineType.Activation
    POOL = mybir.EngineType.Pool

    chains = {}

    def link(key, instr, reason="order"):
        prev = chains.get(key)
        if prev is not None:
            tile.add_dep_helper(instr.ins, prev.ins, reason=reason)
        chains[key] = instr
        return instr

    def loads(engine, s, cols):
        li, vals = nc.values_load_multi_w_load_instructions(
            poff_sb[0:1, s, cols[0] : cols[0] + len(cols)],
            engines=[engine],
            min_val=0,
            max_val=6,
            skip_runtime_bounds_check=True,
        )
        return li, vals

    def loads1(engine, s, col):
        return nc.values_load(
            poff_sb[0:1, s, col : col + 1],
            engines=[engine],
            min_val=0,
            max_val=6,
            skip_runtime_bounds_check=True,
        )

    with tile.TileContext(nc) as tc, ExitStack() as ctx:
        psum_pool = ctx.enter_context(tc.tile_pool(name="psum", bufs=8, space="PSUM"))

        # ---- PE warmup ----
        pw = psum_pool.tile([128, NT], F32, tag="ps", name="pw")
        for i in range(NWARM):
            link("pe", nc.tensor.matmul(
                out=pw[:], lhsT=wscr[:, 0:128], rhs=wscr[:, 128 : 128 + NT],
                start=True, stop=True,
            ))

        # ---- DMAs ----
        nc.gpsimd.dma_start(out=params_sb, in_=params)
        # SP queue: xin in consumption order
        nc.sync.dma_start(out=XP[0][:, 0], in_=xin[:, 0, 0])
        nc.sync.dma_start(out=XP[0][:, 1], in_=xin[:, 0, 1])
        nc.sync.dma_start(out=XP[1][:, 0], in_=xin[:, 1, 0])
        nc.sync.dma_start(out=XP[1][:, 1], in_=xin[:, 1, 1])
        # ACT queue: filter chunks
        nc.scalar.dma_start(out=filt_sb[:, 0, 0:2, :], in_=filt[:, 0, 0:2, :])
        nc.scalar.dma_start(out=filt_sb[:, 0, 2:6, :], in_=filt[:, 0, 2:6, :])
        nc.scalar.dma_start(out=filt_sb[:, 0, 6:18, :], in_=filt[:, 0, 6:18, :])
        nc.scalar.dma_start(out=filt_sb[:, 1, :, :], in_=filt[:, 1, :, :])

        # ---- register loads (s0 only; s1 loads deferred below) ----
        dve_vals, act_vals, pool_vals = {}, {}, {}
        _, dve_vals[0] = loads(DVE, 0, (0, 1, 2, 3))   # e0, e2, d0, d2
        _, act_vals[0] = loads(ACT, 0, (2, 3))         # d0, d2
        _, pool_vals[0] = loads(POOL, 0, (2, 3))


        # ---- DVE stages: s0 A, s0 B, s1 A, s1 B ----
        for s in range(2):
            if s == 1:
                _, dve_vals[1] = loads(DVE, 1, (0, 1, 2, 3))
            e0, e2, d0, d2 = dve_vals[s]
            for b in range(2):
                for j, e, rc in ((0, e0, 0), (2, e2, 1)):
                    r = pratio_sb[:, s, rc : rc + 1]
                    sl = (0 if j == 0 else 2) + b
                    link("dve", nc.vector.tensor_scalar(
                        out=CI[s][:, sl], in0=XP[s][:, b, :, bass.ds(e + 1, H)],
                        scalar1=r, scalar2=None, op0=MULT,
                    ))
                    link("dve", nc.vector.tensor_tensor(
                        out=CI[s][:, sl], in0=CI[s][:, sl],
                        in1=XP[s][:, b, :, bass.ds(e, H)], op=ADD,
                    ))
            for il, (i, d, rc) in enumerate(((0, d0, 2), (2, d2, 3))):
                ri = pratio_sb[:, s, rc : rc + 1]
                for lo, hi in ((0, 2), (2, 4)):
                    link("dve", nc.vector.tensor_scalar(
                        out=XO[s][:, il, lo:hi],
                        in0=CI[s][:, lo:hi, bass.ds(d + 1, H), :],
                        scalar1=ri, scalar2=None, op0=MULT,
                    ))
                    link("dve", nc.vector.tensor_tensor(
                        out=XO[s][:, il, lo:hi], in0=XO[s][:, il, lo:hi],
                        in1=CI[s][:, lo:hi, bass.ds(d, H), :], op=ADD,
                    ))

        # ---- j=1 slabs: ACT copy + Pool add, s0 then s1 ----
        for s in range(2):
            if s == 1:
                _, act_vals[1] = loads(ACT, 1, (2, 3))
                _, pool_vals[1] = loads(POOL, 1, (2, 3))
            da0, da2 = act_vals[s]
            dp0, dp2 = pool_vals[s]
            for il, (da, dp, rc) in enumerate(((da0, dp0, 2), (da2, dp2, 3))):
                ri = pratio_sb[:, s, rc : rc + 1]
                for b in range(2):
                    sl = XO[s][:, il, 4 + b]
                    link("act", nc.scalar.activation(
                        out=sl, in_=XP[s][:, b, bass.ds(da + 1, H), PAD : PAD + H],
                        func=COPY, scale=ri,
                    ))
                    link("pool", nc.gpsimd.tensor_tensor(
                        out=sl, in0=sl,
                        in1=XP[s][:, b, bass.ds(dp, H), PAD : PAD + H], op=ADD,
                    ))

        # ---- conv ----
        def rhs_view(s, tap, b, n):
            i, j = tap // 3, tap % 3
            rows = slice(n * NROW, (n + 1) * NROW)
            prows = slice(PAD + n * NROW, PAD + (n + 1) * NROW)
            if i == 1:
                if j == 1:
                    return XP[s][:, b, prows, PAD : PAD + H]
                return CI[s][:, (0 if j == 0 else 2) + b, prows, :]
            il = 0 if i == 0 else 1
            sl = (4 if j == 1 else (0 if j == 0 else 2)) + b
            return XO[s][:, il, sl, rows, :]

        KTSEQ = [(4, 0), (4, 1), (3, 0), (5, 0), (3, 1), (5, 1),
                 (1, 0), (1, 1), (0, 0), (0, 1), (2, 0), (2, 1), (7, 0), (7, 1)]
        KTIDX = {t: i for i, t in enumerate(TAPORD)}

        for s in range(2):
            # final psum of the kernel (s1, o1, n2) is split 288+144 so the
            # very last evac+DMA chain is short
            split_last = s == 1
            psums = [[None] * NN for _ in range(2)]
            nsplit = []
            for o in range(2):
                for n in range(NN):
                    if split_last and o == 1 and n == NN - 1:
                        psums[o][n] = (
                            psum_pool.tile([128, 288], F32, tag="ps", name=f"ps_{s}_{o}_{n}a"),
                            psum_pool.tile([128, 144], F32, tag="ps", name=f"ps_{s}_{o}_{n}b"),
                        )
                    else:
                        psums[o][n] = psum_pool.tile(
                            [128, NT], F32, tag="ps", name=f"ps_{s}_{o}_{n}"
                        )

            def mm(o, n, tap, b, first, stop):
                kt = 2 * KTIDX[tap] + b
                lhsT = filt_sb[:, s, kt, o * 128 : (o + 1) * 128]
                rv = rhs_view(s, tap, b, n)
                ps = psums[o][n]
                if isinstance(ps, tuple):
                    ra, rb = rv.split_free(288) if hasattr(rv, "split_free") else (None, None)
                    link("pe", nc.tensor.matmul(
                        out=ps[0][:], lhsT=lhsT, rhs=rv[:, 0:8, :],
                        start=first, stop=stop,
                    ))
                    link("pe", nc.tensor.matmul(
                        out=ps[1][:], lhsT=lhsT, rhs=rv[:, 8:12, :],
                        start=first, stop=stop,
                    ))
                else:
                    link("pe", nc.tensor.matmul(
                        out=ps[:], lhsT=lhsT, rhs=rv, start=first, stop=stop,
                    ))

            for ki, (tap, b) in enumerate(KTSEQ):
                for o in range(2):
                    for n in range(NN):
                        mm(o, n, tap, b, ki == 0, False)
            gidx = 0
            # stop-group order: for s1, finish (o1,n2a) first and the tiny
            # (o1,n2b) psum last so only its short evac+DMA chain trails the
            # final matmul; its DMA launches from the idle ACT queue.
            if split_last:
                # (o, n, half, evac_engine, dma_queue)
                group_order = [(1, 2, "a", "act", "sp"), (0, 0, None, "dve", "sp"),
                               (0, 1, None, "act", "actq"), (0, 2, None, "dve", "sp"),
                               (1, 0, None, "act", "sp"), (1, 1, None, "dve", "actq"),
                               (1, 2, "b", "dve", "sp")]
            else:
                group_order = [(o, n, None, "act", "sp") for o in range(2) for n in range(NN)]
            for o, n, half, ev, dq in group_order:
                ps = psums[o][n]
                sub = isinstance(ps, tuple)
                for tap, b in ((6, 0), (6, 1), (8, 0), (8, 1)):
                    kt = 2 * KTIDX[tap] + b
                    lhsT = filt_sb[:, s, kt, o * 128 : (o + 1) * 128]
                    rv = rhs_view(s, tap, b, n)
                    stop = tap == 8 and b == 1
                    if sub and half == "a":
                        link("pe", nc.tensor.matmul(
                            out=ps[0][:], lhsT=lhsT, rhs=rv[:, 0:8, :],
                            start=False, stop=stop,
                        ))
                    elif sub and half == "b":
                        link("pe", nc.tensor.matmul(
                            out=ps[1][:], lhsT=lhsT, rhs=rv[:, 8:12, :],
                            start=False, stop=stop,
                        ))
                    else:
                        link("pe", nc.tensor.matmul(
                            out=ps[:], lhsT=lhsT, rhs=rv, start=False, stop=stop,
                        ))
                if sub and half == "a":
                    osl, ysl = OSB[s][:, o, n, 0:288], yout[
                        s, o * 128 : (o + 1) * 128, n * NROW : n * NROW + 8, :]
                    src_ps = ps[0]
                elif sub and half == "b":
                    osl, ysl = OSB[s][:, o, n, 288:NT], yout[
                        s, o * 128 : (o + 1) * 128, n * NROW + 8 : (n + 1) * NROW, :]
                    src_ps = ps[1]
                else:
                    osl, ysl = OSB[s][:, o, n], yout[
                        s, o * 128 : (o + 1) * 128, n * NROW : (n + 1) * NROW, :]
                    src_ps = ps
                if ev == "act":
                    link("act", nc.scalar.activation(out=osl, in_=src_ps[:], func=COPY))
                else:
                    link("dve", nc.vector.tensor_scalar(
                        out=osl, in0=src_ps[:], scalar1=1.0, scalar2=None, op0=MULT,
                    ))
                (nc.sync if dq == "sp" else nc.scalar).dma_start(out=ysl, in_=osl)
                gidx += 1
    if not nc.is_finalized():
        nc.finalize()
    return nc


def _get_nc():
    global _CACHED_NC
    if _CACHED_NC is None:
        _CACHED_NC = _build_nc()
    return _CACHED_NC


def _sample_params(off_b):
    prm = {}
    for axis in (0, 1):
        s = np.float32(KS) / np.float32(off_b[axis])
        per = {}
        for i, rr in ((0, np.float32(-1.0)), (2, np.float32(1.0))):
            d = rr * s
            D = int(np.floor(d))
            f = np.float32(d - np.float32(D))
            per[i] = (D, f, np.float32(f / (np.float32(1.0) - f)), np.float32(1.0) - f)
        prm[axis] = per
    return prm


def kernel(x, target_filter, offset):
    x = np.ascontiguousarray(np.asarray(x, dtype=np.float32))
    tf = np.ascontiguousarray(np.asarray(target_filter, dtype=np.float32))
    offset = np.asarray(offset, dtype=np.float32)

    nc = _get_nc()

    tfg = [
        np.ascontiguousarray(
            tf[g * OG : (g + 1) * OG].reshape(OG, 2, 128, KS, KS)
            .transpose(3, 4, 1, 2, 0)  # [i, j, b, 128p, 256o]
        )
        for g in range(NSQ)
    ]

    in_maps = []
    core_meta = []
    for k in range(8):
        g = k % 4
        ms = (2 * (k // 4), 2 * (k // 4) + 1)
        bs = [m * NSQ + g for m in ms]
        xin = np.zeros((128, 2, 2, HP, HP), NPBF)
        for si in range(2):
            xs = x[ms[si], g]
            for b in range(2):
                xin[:, si, b, PAD : PAD + H, PAD : PAD + H] = (
                    xs[b * 128 : (b + 1) * 128].astype(NPBF)
                )

        filt = np.zeros((128, 2, 18, OG), NPBF)
        pratio = np.zeros((2, 4), np.float32)
        poff = np.zeros((2, 4), np.int32)
        for si, b in enumerate(bs):
            prm = _sample_params(offset[b])
            rows, cols = prm[0], prm[1]
            poff[si, 0] = cols[0][0] + PAD
            poff[si, 1] = cols[2][0] + PAD
            poff[si, 2] = rows[0][0] + PAD
            poff[si, 3] = rows[2][0] + PAD
            pratio[si] = [cols[0][2], cols[2][2], rows[0][2], rows[2][2]]
            for tidx, tap in enumerate(TAPORD):
                i, j = tap // 3, tap % 3
                sc = np.float32(
                    (1.0 if i == 1 else rows[i][3]) * (1.0 if j == 1 else cols[j][3])
                )
                for bb in range(2):
                    filt[:, si, 2 * tidx + bb, :] = (
                        tfg[g][i, j, bb].astype(np.float32) * sc
                    ).astype(NPBF)
        assert poff.min() >= 0 and poff.max() <= 6, poff
        params = np.zeros((128, 2, 8), np.float32)
        params[:, :, 0:4] = pratio[None]
        params[:, :, 4:8] = poff[None].view(np.float32)
        in_maps.append({"xin": xin, "filt": filt, "params": params})
        core_meta.append((g, ms))

    trace = bool(int(os.environ.get("KERNEL_TRACE", "0")))
    res = None
    last_exc = None
    for attempt in range(3):
        try:
            res = run_bass_kernel_spmd(
                nc, in_maps, list(range(8)), trace=trace and attempt == 0
            )
            break
        except Exception as exc:
            last_exc = exc
    if res is None:
        raise last_exc
    global LAST_RESULTS
    LAST_RESULTS = res

    out = np.empty((NIMG, NSQ * OG, H, H), np.float32)
    for k in range(8):
        g, ms = core_meta[k]
        y = res.results[k]["yout"]
        for si, m in enumerate(ms):
            out[m, g * OG : (g + 1) * OG] = np.asarray(y[si]).astype(np.float32)
    return out

